# revision 1
# baseline (speedup 1.0000x reference)
"""Cross-attention kernel for 8 TRN2 NeuronCores.

Reference shapes: x [4, 2048, 1024], embeds [4, 2048, 1024],
Wq/Wk/Wv [1024, 1024] (+bias), Wo [1024, 1024] (+bias), H=16 heads, D=64.

Sharding: core c handles batch b = c//2 and head group hg = c%2 (8 heads,
attn-dim slice of 512).  Each core computes a partial output
outT_c [1024, 2048] = (ctx_c @ Wo[hg-slice]) ^T; the host sums the two
partials per batch (row-parallel Wo all-reduce done at unshard time) and
adds nothing else (bo is folded into the even core's partial).

Device dataflow per core (activations kept feature-major, "T" = [feat, tok]):
  QT = Wq_c^T @ xT      [512, 2048]   (fp32r matmuls, psum fp32)
  KT = Wk_c^T @ embT    [512, 2048]
  V  = embT^T-proj      [2048, 512]   token-major, + ones column per head
  per head h, lq-half: ST = K_h @ Q_h^T  -> exp (ACT, scale=1/8) -> E
                       [C';denom] = [V_h|1]^T @ E   (ones-column trick)
                       CT_h = C' * (1/denom)  (recip + partition_broadcast)
  outT = Wo_c^T @ CT    [1024, 2048]  + bo (even core only)
Softmax skips the max-subtraction: scores ~ N(0,1), |s| < ~7, exp is safe
in fp32 and matches the reference softmax mathematically.
"""

import os
import sys

if "/opt/trn_rl_repo" not in sys.path:
    sys.path.insert(0, "/opt/trn_rl_repo")

import numpy as np

import concourse.bass as bass  # noqa: F401  (engine namespaces live on nc)
import concourse.mybir as mybir
import concourse.tile as tile
from concourse import bacc
from concourse.bass_utils import run_bass_kernel_spmd

P = 128
B, LQ, LK, DIM = 4, 2048, 2048, 1024
H, D = 16, 64
ADC = 512          # per-core attention dim (8 heads x 64)
NHC = 8            # heads per core
SCALE = 1.0 / 8.0
F32 = mybir.dt.float32
FR = mybir.dt.float32r
EXP = mybir.ActivationFunctionType.Exp

K_T = DIM // P     # 8 contraction tiles for projections
M_AD = ADC // P    # 4 ad partition tiles
T_LK = LK // P     # 16 lk tiles
VW = NHC * (D + 1)  # 520: V block width per lk tile (64 cols + ones col per head)

_CACHE = {}
_PHASES = int(os.environ.get("KPHASES", "3"))


def _build():
    nc = bacc.Bacc("TRN2", target_bir_lowering=False, debug=False)

    xT = nc.dram_tensor("xT", [DIM, LQ], FR, kind="ExternalInput").ap()
    embT = nc.dram_tensor("embT", [DIM, LK], FR, kind="ExternalInput").ap()
    Wq = nc.dram_tensor("Wq", [DIM, ADC], FR, kind="ExternalInput").ap()
    Wk = nc.dram_tensor("Wk", [DIM, ADC], FR, kind="ExternalInput").ap()
    Wv = nc.dram_tensor("Wv", [DIM, ADC], FR, kind="ExternalInput").ap()
    Wo = nc.dram_tensor("Wo", [ADC, DIM], FR, kind="ExternalInput").ap()
    bq = nc.dram_tensor("bq", [P, M_AD], F32, kind="ExternalInput").ap()
    bk = nc.dram_tensor("bk", [P, M_AD], F32, kind="ExternalInput").ap()
    bvb = nc.dram_tensor("bvb", [P, ADC], F32, kind="ExternalInput").ap()
    bo = nc.dram_tensor("bo", [P, DIM // P], F32, kind="ExternalInput").ap()
    outT = nc.dram_tensor("outT", [DIM, LQ], F32, kind="ExternalOutput").ap()

    with tile.TileContext(nc) as tc:
        with tc.tile_pool(name="resident", bufs=1) as res:
            QT = [res.tile([P, LQ], FR, name=f"qt{m}") for m in range(M_AD)]
            KT = [res.tile([P, LK], FR, name=f"kt{m}") for m in range(M_AD)]
            V = res.tile([P, T_LK * VW], FR, name="v")
            CT = [res.tile([P, LQ], FR, name=f"ct{p}") for p in range(M_AD)]
            WO = res.tile([P, ADC // P, DIM], FR, name="wo")
            bq_sb = res.tile([P, M_AD], F32, name="bq")
            bk_sb = res.tile([P, M_AD], F32, name="bk")
            bvb_sb = res.tile([P, ADC], F32, name="bvb")
            bo_sb = res.tile([P, DIM // P], F32, name="bo")

            nc.sync.dma_start(WO[:], Wo.rearrange("(k p) n -> p k n", p=P))
            nc.sync.dma_start(bq_sb[:], bq[:])
            nc.sync.dma_start(bk_sb[:], bk[:])
            nc.sync.dma_start(bvb_sb[:], bvb[:])
            nc.sync.dma_start(bo_sb[:], bo[:])

            # ones columns (col 64 of each head's 65-wide block) for the
            # fused-denominator C matmul.  memset can't write fp32r, so
            # synthesize 1.0 on DVE as in0*0 + 1.
            zsrc = res.tile([P, NHC], F32, name="zsrc")
            nc.gpsimd.memset(zsrc[:], 0.0)
            for t in range(T_LK):
                blk = V[:, t * VW:(t + 1) * VW].rearrange(
                    "p (a b) -> p a b", b=D + 1)
                nc.vector.tensor_scalar(
                    blk[:, :, D:D + 1],
                    zsrc[:].rearrange("p (a b) -> p a b", b=1),
                    0.0, 1.0,
                    op0=mybir.AluOpType.mult, op1=mybir.AluOpType.add)

            # ---------------- projections ----------------
            # V first (attention needs all of V), then Q/K pair-by-pair so
            # attention on pair 0 can start while later pairs project.
            with tc.tile_pool(name="wproj", bufs=2) as wpool, \
                 tc.tile_pool(name="stream", bufs=4) as spool, \
                 tc.tile_pool(name="pjp", bufs=1, space="PSUM") as pjp, \
                 tc.tile_pool(name="pjv", bufs=2, space="PSUM") as pjv:

                wv_sb = wpool.tile([P, K_T, ADC], FR, name="w")
                for k in range(K_T):
                    nc.sync.dma_start(wv_sb[:, k, :],
                                      Wv[k * P:(k + 1) * P, :])
                embT_kp = embT.rearrange("(k p) n -> p k n", p=P)
                for t in range(T_LK):
                    vk = spool.tile([P, K_T, P], FR, name="vk")
                    nc.sync.dma_start(
                        vk[:], embT_kp[:, :, t * P:(t + 1) * P])
                    psv = pjv.tile([P, ADC], F32, name="pv")
                    for k in range(K_T):
                        nc.tensor.matmul(psv[:], vk[:, k, :], wv_sb[:, k, :],
                                         start=(k == 0), stop=(k == K_T - 1))
                    vdst = V[:, t * VW:(t + 1) * VW].rearrange(
                        "p (a b) -> p a b", b=D + 1)[:, :, 0:D]
                    nc.vector.tensor_tensor(
                        vdst,
                        psv[:].rearrange("p (a b) -> p a b", b=D),
                        bvb_sb[:].rearrange("p (a b) -> p a b", b=D),
                        op=mybir.AluOpType.add)

                for (w_dram, b_sb, out_tiles, src) in (
                        (Wq, bq_sb, QT, xT), (Wk, bk_sb, KT, embT)):
                    w_sb = wpool.tile([P, K_T, ADC], FR, name="w")
                    for k in range(K_T):
                        nc.sync.dma_start(w_sb[:, k, :],
                                          w_dram[k * P:(k + 1) * P, :])
                    for n in range(LQ // 512):
                        pps = [pjp.tile([P, 512], F32, name=f"pp{m}")
                               for m in range(M_AD)]
                        for k in range(K_T):
                            xt = spool.tile([P, 512], FR, name="xs")
                            nc.sync.dma_start(
                                xt[:],
                                src[k * P:(k + 1) * P, n * 512:(n + 1) * 512])
                            for m in range(M_AD):
                                nc.tensor.matmul(
                                    pps[m][:],
                                    w_sb[:, k, m * P:(m + 1) * P],
                                    xt[:],
                                    start=(k == 0), stop=(k == K_T - 1))
                        for m in range(M_AD):
                            nc.vector.tensor_scalar_add(
                                out_tiles[m][:, n * 512:(n + 1) * 512],
                                pps[m][:], b_sb[:, m:m + 1])

            # ---------------- attention ----------------
            # Head pairs interleaved: the two heads of a pair occupy PE row
            # groups 0-63 / 64-127 (tile_position auto-derived from the
            # base partition), so their K=64 S-matmuls run concurrently.
            with tc.tile_pool(name="aps", bufs=1, space="PSUM") as aps, \
                 tc.tile_pool(name="apc", bufs=1, space="PSUM") as apc, \
                 tc.tile_pool(name="etp", bufs=2) as etp, \
                 tc.tile_pool(name="small", bufs=1) as small:
                for p in range(M_AD if _PHASES >= 2 else 0):
                    mt = p
                    for half in range(2):
                        q0 = half * 1024
                        pcs = [apc.tile([D + 1, 1024], F32, name=f"pc{a}")
                               for a in range(2)]
                        for t in range(T_LK):
                            pss = []
                            for a in range(2):
                                ro = a * D
                                ps = aps.tile([P, 1024], F32, name=f"ps{a}")
                                for nn in range(2):
                                    nc.tensor.matmul(
                                        ps[:, nn * 512:(nn + 1) * 512],
                                        KT[mt][ro:ro + D, t * P:(t + 1) * P],
                                        QT[mt][ro:ro + D,
                                               q0 + nn * 512:
                                               q0 + (nn + 1) * 512],
                                        start=True, stop=True)
                                pss.append(ps)
                            ets = []
                            for a in range(2):
                                et = etp.tile([P, 1024], FR, name=f"et{a}")
                                nc.scalar.activation(et[:], pss[a][:], EXP,
                                                     scale=SCALE)
                                ets.append(et)
                            for a in range(2):
                                vcol = (2 * p + a) * (D + 1)
                                for nn in range(2):
                                    nc.tensor.matmul(
                                        pcs[a][:, nn * 512:(nn + 1) * 512],
                                        V[:, t * VW + vcol:
                                           t * VW + vcol + D + 1],
                                        ets[a][:, nn * 512:(nn + 1) * 512],
                                        start=(t == 0), stop=(t == T_LK - 1))
                        for a in range(2):
                            ro = a * D
                            r1 = small.tile([1, 1024], F32, name=f"r1{a}")
                            nc.vector.reciprocal(r1[:], pcs[a][D:D + 1, :])
                            rb = small.tile([D, 1024], F32, name=f"rb{a}")
                            nc.gpsimd.partition_broadcast(rb[:], r1[0:1, :])
                            nc.vector.tensor_tensor(
                                CT[mt][ro:ro + D, q0:q0 + 1024],
                                pcs[a][0:D, :], rb[:],
                                op=mybir.AluOpType.mult)

            # ---------------- output projection ----------------
            with tc.tile_pool(name="ops", bufs=4, space="PSUM") as ops, \
                 tc.tile_pool(name="ostage", bufs=4) as ostage:
                for m in range(DIM // P if _PHASES >= 3 else 0):
                    for n in range(LQ // 512):
                        po = ops.tile([P, 512], F32, name="po")
                        for kk in range(ADC // P):
                            nc.tensor.matmul(
                                po[:],
                                WO[:, kk, m * P:(m + 1) * P],
                                CT[kk][:, n * 512:(n + 1) * 512],
                                start=(kk == 0), stop=(kk == ADC // P - 1))
                        ot = ostage.tile([P, 512], F32, name="ot")
                        nc.vector.tensor_scalar_add(ot[:], po[:],
                                                    bo_sb[:, m:m + 1])
                        nc.sync.dma_start(
                            outT[m * P:(m + 1) * P, n * 512:(n + 1) * 512],
                            ot[:])

    nc.compile()
    return nc


def _in_maps(x, embeds, Wq, bq, Wk, bk, Wv, bv, Wo, bo):
    f = np.float32
    maps = []
    for c in range(8):
        b, hg = c // 2, c % 2
        s = slice(hg * ADC, (hg + 1) * ADC)
        bo_c = bo if hg == 0 else np.zeros_like(bo)
        maps.append({
            "xT": np.ascontiguousarray(x[b].T, dtype=f),
            "embT": np.ascontiguousarray(embeds[b].T, dtype=f),
            "Wq": np.ascontiguousarray(Wq[:, s], dtype=f),
            "Wk": np.ascontiguousarray(Wk[:, s], dtype=f),
            "Wv": np.ascontiguousarray(Wv[:, s], dtype=f),
            "Wo": np.ascontiguousarray(Wo[s, :], dtype=f),
            "bq": np.ascontiguousarray(
                bq[s].reshape(M_AD, P).T, dtype=f),
            "bk": np.ascontiguousarray(
                bk[s].reshape(M_AD, P).T, dtype=f),
            "bvb": np.ascontiguousarray(
                np.tile(bv[s], (P, 1)), dtype=f),
            "bo": np.ascontiguousarray(
                bo_c.reshape(DIM // P, P).T, dtype=f),
        })
    return maps


def kernel(x, embeds, Wq, bq, Wk, bk, Wv, bv, Wo, bo, _trace=False,
           _tmpdir=None):
    x = np.asarray(x); embeds = np.asarray(embeds)
    Wq = np.asarray(Wq); bq = np.asarray(bq)
    Wk = np.asarray(Wk); bk = np.asarray(bk)
    Wv = np.asarray(Wv); bv = np.asarray(bv)
    Wo = np.asarray(Wo); bo = np.asarray(bo)

    if "nc" not in _CACHE:
        _CACHE["nc"] = _build()
    nc = _CACHE["nc"]

    maps = _in_maps(x, embeds, Wq, bq, Wk, bk, Wv, bv, Wo, bo)
    res = run_bass_kernel_spmd(nc, maps, core_ids=list(range(8)),
                               trace=_trace, tmpdir=_tmpdir)
    if _trace:
        _CACHE["last_exec_time_ns"] = res.exec_time_ns
        _CACHE["last_results"] = res

    out = np.empty((B, LQ, DIM), np.float32)
    for b in range(B):
        acc = res.results[2 * b]["outT"] + res.results[2 * b + 1]["outT"]
        out[b] = acc.T
    return out



# revision 13
# speedup vs baseline: 1.1073x; 1.1073x over previous
"""Cross-attention kernel for 8 TRN2 NeuronCores.

Reference shapes: x [4, 2048, 1024], embeds [4, 2048, 1024],
Wq/Wk/Wv [1024, 1024] (+bias), Wo [1024, 1024] (+bias), H=16 heads, D=64.

Sharding: core c handles batch b = c//2 and head group hg = c%2 (8 heads,
attn-dim slice of 512).  Each core computes a partial output
outT_c [1024, 2048] (fp16); the host sums the two partials per batch
(row-parallel Wo all-reduce done at unshard time); bo is folded into the
even core's partial.

All matmul operands are fp16 (PSUM accumulates fp32).  Device dataflow:
  QT[m] = Wq_m^T @ xT      [128, 2048] per ad-tile m (4)   feature-major
  KT[m] = Wk_m^T @ embT    [128, 2048]
  V[t]  = embT_t^T @ Wv    [128, 520]  token-major, 8 heads x (64 cols + ones)
  per head h, q-half qh (1024 q):
    per lk-tile t: S = K_h-slice^T-form @ Q_h -> psum [128 lk, 1024 q]
                   E = exp(S/8)               -> sbuf fp16 (ACT, 1024-wide)
                   Cu[qc] += E_chunk^T @ [V_h|1]   psum [128 q, 65] per qc
    normalize: ctx_tok = Cu[:, :64] / Cu[:, 64]  (DVE divide, per-q scalar)
  transpose ctx_tok [q, ad] -> CT [ad, q] via XBAR DMA transpose (fp16)
  outT = Wo^T @ CT  + bo (even core)    -> fp16 out
Softmax skips the max-subtraction: scores ~ N(0,1), exp is safe in fp32.

Scheduling: engines execute their queues in order, so emission order is the
schedule.  The whole kernel is one "tick" loop over the 256 exp tiles
(qh, head, lk-tile): each tick emits the tile's S matmuls + exp, the ctx
matmuls of the tile two back (E-ring), any projection units whose deadline
arrived, and a budgeted trickle of remaining projection / output work.
This keeps the activation engine's exp stream (the ~266us floor) running
back-to-back while the PE (the ~274us floor) stays saturated.  The ctx
matmul is token-major (65-wide moving operand) because a matmul costs its
output free size: 8 heads x 16 q-chunks x 16 lk-passes x 65 halves the PE
cost vs the feature-major form.
"""

import sys

if "/opt/trn_rl_repo" not in sys.path:
    sys.path.insert(0, "/opt/trn_rl_repo")

import numpy as np

import concourse.bass as bass  # noqa: F401
import concourse.mybir as mybir
import concourse.tile as tile
from concourse import bacc
from concourse.bass_utils import run_bass_kernel_spmd

P = 128
B, LQ, LK, DIM = 4, 2048, 2048, 1024
H, D = 16, 64
ADC = 512          # per-core attention dim (8 heads x 64)
NHC = 8            # heads per core
SCALE = 1.0 / 8.0
F32 = mybir.dt.float32
F16 = mybir.dt.float16
EXP = mybir.ActivationFunctionType.Exp

K_T = DIM // P     # 8 contraction tiles for projections
M_AD = ADC // P    # 4 ad partition tiles (head pairs)
T_LK = LK // P     # 16 lk tiles
VW = D + 1         # 65: per-head V block width (64 cols + ones col)
VTW = NHC * VW     # 520: V block width per lk tile
LAG = 3            # ctx matmuls trail exp by this many ticks
ERING = 6          # E-ring depth (sbuf fp16 [128, 1024] slots)

_CACHE = {}


def _build():
    nc = bacc.Bacc("TRN2", target_bir_lowering=False, debug=False)

    xT = nc.dram_tensor("xT", [DIM, LQ], F16, kind="ExternalInput").ap()
    embT = nc.dram_tensor("embT", [DIM, LK], F16, kind="ExternalInput").ap()
    Wq = nc.dram_tensor("Wq", [DIM, ADC], F16, kind="ExternalInput").ap()
    Wk = nc.dram_tensor("Wk", [DIM, ADC], F16, kind="ExternalInput").ap()
    Wv = nc.dram_tensor("Wv", [DIM, ADC], F16, kind="ExternalInput").ap()
    Wo = nc.dram_tensor("Wo", [ADC, DIM], F16, kind="ExternalInput").ap()
    bq = nc.dram_tensor("bq", [P, M_AD], F32, kind="ExternalInput").ap()
    bk = nc.dram_tensor("bk", [P, M_AD], F32, kind="ExternalInput").ap()
    bvb = nc.dram_tensor("bvb", [P, ADC], F32, kind="ExternalInput").ap()
    bo = nc.dram_tensor("bo", [P, DIM // P], F32, kind="ExternalInput").ap()
    outT = nc.dram_tensor("outT", [DIM, LQ], F16, kind="ExternalOutput").ap()

    with tile.TileContext(nc) as tc:
        with tc.tile_pool(name="resident", bufs=1) as res:
            xs = res.tile([P, K_T, LQ], F16, name="xs")
            es = res.tile([P, K_T, LK], F16, name="es")
            wq_sb = res.tile([P, K_T, ADC], F16, name="wq")
            wk_sb = res.tile([P, K_T, ADC], F16, name="wk")
            wv_sb = res.tile([P, K_T, ADC], F16, name="wv")
            wo_sb = res.tile([P, M_AD, DIM], F16, name="wo")
            QT = [res.tile([P, LQ], F16, name=f"qt{m}") for m in range(M_AD)]
            KT = [res.tile([P, LK], F16, name=f"kt{m}") for m in range(M_AD)]
            V = res.tile([P, T_LK * VTW], F16, name="v")
            CT = [res.tile([P, LQ], F16, name=f"ct{m}") for m in range(M_AD)]
            bq_sb = res.tile([P, M_AD], F32, name="bq")
            bk_sb = res.tile([P, M_AD], F32, name="bk")
            bvb_sb = res.tile([P, ADC], F32, name="bvb")
            bo_sb = res.tile([P, DIM // P], F32, name="bo")

            # ---- input DMAs (SP queue; loads only, never block) ----
            # Ordered so the first S matmul (needs K pair 0 chunk 0 + Q pair
            # 0 q-half 0) can launch as early as possible.
            nc.sync.dma_start(bq_sb[:], bq[:])
            nc.sync.dma_start(bk_sb[:], bk[:])
            nc.sync.dma_start(bo_sb[:], bo[:])
            nc.sync.dma_start(bvb_sb[:], bvb[:])
            nc.sync.dma_start(wk_sb[:], Wk.rearrange("(k p) n -> p k n", p=P))
            nc.sync.dma_start(wq_sb[:], Wq.rearrange("(k p) n -> p k n", p=P))
            embT_kp = embT.rearrange("(k p) n -> p k n", p=P)
            xT_kp = xT.rearrange("(k p) n -> p k n", p=P)
            nc.sync.dma_start(es[:, :, 0:512], embT_kp[:, :, 0:512])
            nc.sync.dma_start(xs[:, :, 0:512], xT_kp[:, :, 0:512])
            nc.sync.dma_start(xs[:, :, 512:1024], xT_kp[:, :, 512:1024])
            nc.sync.dma_start(wv_sb[:], Wv.rearrange("(k p) n -> p k n", p=P))
            for n in range(1, 4):
                nc.sync.dma_start(es[:, :, n * 512:(n + 1) * 512],
                                  embT_kp[:, :, n * 512:(n + 1) * 512])
            nc.sync.dma_start(xs[:, :, 1024:2048], xT_kp[:, :, 1024:2048])
            nc.sync.dma_start(wo_sb[:], Wo.rearrange("(k p) n -> p k n", p=P))

            # ones columns for the fused-denominator ctx matmul: preset the
            # whole V tile to 1.0; V-proj bias-add overwrites the 64-wide
            # value blocks and leaves column 64 of each head block intact.
            nc.gpsimd.memset(V[:], 1.0)

            with tc.tile_pool(name="pj", bufs=2, space="PSUM") as pjp, \
                 tc.tile_pool(name="sw", bufs=2, space="PSUM") as swp, \
                 tc.tile_pool(name="cp", bufs=1, space="PSUM") as cpp, \
                 tc.tile_pool(name="ep", bufs=ERING) as epp, \
                 tc.tile_pool(name="ctok", bufs=2) as ctokp, \
                 tc.tile_pool(name="rcp", bufs=2) as rcpp, \
                 tc.tile_pool(name="os", bufs=4) as osp:

                # ---------- emission helpers ----------
                def emit_kproj(m, n):
                    ps = pjp.tile([P, 512], F32, name="pp")
                    for k in range(K_T):
                        nc.tensor.matmul(
                            ps[:], wk_sb[:, k, m * P:(m + 1) * P],
                            es[:, k, n * 512:(n + 1) * 512],
                            start=(k == 0), stop=(k == K_T - 1))
                    nc.vector.tensor_scalar_add(
                        KT[m][:, n * 512:(n + 1) * 512], ps[:],
                        bk_sb[:, m:m + 1])

                def emit_qproj(m, n):
                    ps = pjp.tile([P, 512], F32, name="pp")
                    for k in range(K_T):
                        nc.tensor.matmul(
                            ps[:], wq_sb[:, k, m * P:(m + 1) * P],
                            xs[:, k, n * 512:(n + 1) * 512],
                            start=(k == 0), stop=(k == K_T - 1))
                    nc.vector.tensor_scalar_add(
                        QT[m][:, n * 512:(n + 1) * 512], ps[:],
                        bq_sb[:, m:m + 1])

                def emit_vproj(t):
                    ps = pjp.tile([P, 512], F32, name="pp")
                    for k in range(K_T):
                        nc.tensor.matmul(
                            ps[:], es[:, k, t * P:(t + 1) * P],
                            wv_sb[:, k, :],
                            start=(k == 0), stop=(k == K_T - 1))
                    vdst = V[:, t * VTW:(t + 1) * VTW].rearrange(
                        "p (a b) -> p a b", b=VW)
                    nc.vector.tensor_tensor(
                        vdst[:, :, 0:D],
                        ps[:].rearrange("p (a b) -> p a b", b=D),
                        bvb_sb[:].rearrange("p (a b) -> p a b", b=D),
                        op=mybir.AluOpType.add)

                def emit_outproj(d, qn):
                    po = pjp.tile([P, 512], F32, name="pp")
                    for ch in range(M_AD):
                        nc.tensor.matmul(
                            po[:], wo_sb[:, ch, d * P:(d + 1) * P],
                            CT[ch][:, qn * 512:(qn + 1) * 512],
                            start=(ch == 0), stop=(ch == M_AD - 1))
                    ot = osp.tile([P, 512], F16, name="ot")
                    nc.vector.tensor_scalar_add(ot[:], po[:],
                                                bo_sb[:, d:d + 1])
                    nc.sync.dma_start(
                        outT[d * P:(d + 1) * P, qn * 512:(qn + 1) * 512],
                        ot[:])

                # ---------- deferred work with deadlines ----------
                # (deadline_tick, avail_tick, est_ns, fn); queue is kept in
                # deadline order.  Forced when the deadline tick arrives;
                # otherwise trickled in by a per-tick nanosecond budget.
                PROJ_NS = 1707  # 8 passes x 512 cols
                OUT_NS = 853    # 4 passes x 512 cols
                work = []
                for m in range(1, M_AD):
                    t0 = 32 * m
                    work.append((t0, 0, PROJ_NS, lambda m=m: emit_qproj(m, 0)))
                    work.append((t0, 0, PROJ_NS, lambda m=m: emit_qproj(m, 1)))
                    for n in range(4):
                        work.append((t0 + 4 * n, 0, PROJ_NS,
                                     lambda m=m, n=n: emit_kproj(m, n)))
                for m in range(M_AD):
                    t0 = 128 + 32 * m
                    work.append((t0, 0, PROJ_NS, lambda m=m: emit_qproj(m, 2)))
                    work.append((t0, 0, PROJ_NS, lambda m=m: emit_qproj(m, 3)))
                for d in range(DIM // P):
                    for qn in range(2):
                        work.append((254, 134, OUT_NS,
                                     lambda d=d, qn=qn: emit_outproj(d, qn)))
                work.sort(key=lambda w: w[0])

                def pump_work(g, budget_ns):
                    # forced: deadline reached
                    while work and work[0][0] <= g:
                        _, _, _, fn = work.pop(0)
                        fn()
                    # trickle: pace remaining work over remaining ticks
                    remaining = sum(w[2] for w in work)
                    rate = remaining / max(1.0, 254.0 - g)
                    spent = 0.0
                    while work and spent < min(budget_ns, rate):
                        for i, (dl, avail, ns, fn) in enumerate(work):
                            if avail <= g:
                                work.pop(i)
                                fn()
                                spent += ns
                                break
                        else:
                            break

                # ---------- startup projections ----------
                emit_kproj(0, 0)
                emit_qproj(0, 0)
                emit_qproj(0, 1)

                # ---------- global tick loop ----------
                ticks = [(qh, m, hh, t)
                         for qh in range(2)
                         for m in range(M_AD)
                         for hh in range(2)
                         for t in range(T_LK)]
                ering = [None] * ERING
                head_state = {}  # h-index -> dict(cu=..., ctok=..., ...)
                cur = {"ctok": None, "qh": -1}

                def cuv(cu, qc):
                    off = (qc // 4) * 512 + (qc % 4) * VW
                    return cu[:, off:off + VW]

                def emit_ctx(hs, t):
                    # ctx matmuls for head-state hs, lk-tile t
                    et = ering[(hs["gbase"] + t) % ERING]
                    cu = hs["cu"]
                    voff = t * VTW + hs["h"] * VW
                    for qc in range(8):
                        # start=True lazily zeroes the whole 2KB psum bank,
                        # so only the first matmul touching each bank sets it
                        nc.tensor.matmul(
                            cuv(cu, qc),
                            et[:, qc * P:(qc + 1) * P],
                            V[:, voff:voff + VW],
                            start=(t == 0 and qc % 4 == 0),
                            stop=(t == T_LK - 1))

                def finish_head(hs):
                    # normalize into ctok; at pair end, XBAR-transpose
                    cu, ctok, h = hs["cu"], hs["ctok"], hs["h"]
                    rcp = rcpp.tile([P, 8], F32, name="rcp")
                    for half in range(2):
                        dn = cu[:, half * 512:half * 512 + 4 * VW].rearrange(
                            "p (a b) -> p a b", b=VW)[:, :, D:D + 1]
                        nc.vector.reciprocal(
                            rcp[:, half * 4:(half + 1) * 4].rearrange(
                                "p (a b) -> p a b", b=1), dn)
                    for qc in range(8):
                        nc.vector.tensor_scalar(
                            ctok[:, qc, h * D:(h + 1) * D],
                            cuv(cu, qc)[:, 0:D],
                            rcp[:, qc:qc + 1], None,
                            op0=mybir.AluOpType.mult)
                    if hs["hh"] == 1:
                        m, q0 = hs["m"], hs["q0"]
                        for qc in range(8):
                            nc.sync.dma_start_transpose(
                                CT[m][:, q0 + qc * P:q0 + (qc + 1) * P],
                                ctok[:, qc, m * P:(m + 1) * P])

                prev_hs = None
                for g, (qh, m, hh, t) in enumerate(ticks):
                    if t == 0:
                        if qh != cur["qh"]:
                            cur["qh"] = qh
                            cur["ctok"] = ctokp.tile(
                                [P, LQ // P // 2, ADC], F16, name="ctok")
                        head_state[g // T_LK] = {
                            "h": 2 * m + hh, "m": m, "hh": hh,
                            "q0": qh * 1024, "gbase": g,
                            "cu": None, "ctok": cur["ctok"],
                        }
                    hs = head_state[g // T_LK]
                    ro = hh * D
                    q0 = qh * 1024

                    # K chunks for pair 0 stream in with the embT DMAs
                    if m == 0 and hh == 0 and qh == 0 and t in (4, 8, 12):
                        emit_kproj(0, t // 4)

                    # S matmuls for this tick's lk-tile
                    sw = swp.tile([P, 1024], F32, name="sw")
                    for nn in range(2):
                        nc.tensor.matmul(
                            sw[:, nn * 512:(nn + 1) * 512],
                            KT[m][ro:ro + D, t * P:(t + 1) * P],
                            QT[m][ro:ro + D,
                                  q0 + nn * 512:q0 + (nn + 1) * 512],
                            start=True, stop=True)
                    et = epp.tile([P, 1024], F16, name="et")
                    nc.scalar.activation(et[:], sw[:], EXP, scale=SCALE)
                    ering[g % ERING] = et

                    # V projections feed head 0's ctx consumption (lag LAG)
                    if qh == 0 and m == 0 and hh == 0 and t < T_LK:
                        emit_vproj(t)

                    # ctx matmuls LAG ticks back
                    gc = g - LAG
                    if gc >= 0:
                        chs = head_state[gc // T_LK]
                        tc_ = gc % T_LK
                        if chs["cu"] is None:
                            chs["cu"] = cpp.tile([P, 1024], F32, name="cu")
                        emit_ctx(chs, tc_)
                        if tc_ == T_LK - 1:
                            finish_head(chs)
                            del head_state[gc // T_LK]

                    pump_work(g, 700)

                # ---------- tail ----------
                for gc in range(256 - LAG, 256):
                    chs = head_state[gc // T_LK]
                    tc_ = gc % T_LK
                    if chs["cu"] is None:
                        chs["cu"] = cpp.tile([P, 1024], F32, name="cu")
                    emit_ctx(chs, tc_)
                    if tc_ == T_LK - 1:
                        finish_head(chs)
                while work:
                    _, _, _, fn = work.pop(0)
                    fn()
                for d in range(DIM // P):
                    emit_outproj(d, 2)
                    emit_outproj(d, 3)

    nc.compile()
    return nc


def _in_maps(x, embeds, Wq, bq, Wk, bk, Wv, bv, Wo, bo):
    h = np.float16
    f = np.float32
    maps = []
    for c in range(8):
        b, hg = c // 2, c % 2
        s = slice(hg * ADC, (hg + 1) * ADC)
        bo_c = bo if hg == 0 else np.zeros_like(bo)
        maps.append({
            "xT": np.ascontiguousarray(x[b].T, dtype=h),
            "embT": np.ascontiguousarray(embeds[b].T, dtype=h),
            "Wq": np.ascontiguousarray(Wq[:, s], dtype=h),
            "Wk": np.ascontiguousarray(Wk[:, s], dtype=h),
            "Wv": np.ascontiguousarray(Wv[:, s], dtype=h),
            "Wo": np.ascontiguousarray(Wo[s, :], dtype=h),
            "bq": np.ascontiguousarray(bq[s].reshape(M_AD, P).T, dtype=f),
            "bk": np.ascontiguousarray(bk[s].reshape(M_AD, P).T, dtype=f),
            "bvb": np.ascontiguousarray(np.tile(bv[s], (P, 1)), dtype=f),
            "bo": np.ascontiguousarray(
                bo_c.reshape(DIM // P, P).T, dtype=f),
        })
    return maps


def kernel(x, embeds, Wq, bq, Wk, bk, Wv, bv, Wo, bo, _trace=False,
           _tmpdir=None):
    x = np.asarray(x); embeds = np.asarray(embeds)
    Wq = np.asarray(Wq); bq = np.asarray(bq)
    Wk = np.asarray(Wk); bk = np.asarray(bk)
    Wv = np.asarray(Wv); bv = np.asarray(bv)
    Wo = np.asarray(Wo); bo = np.asarray(bo)

    if "nc" not in _CACHE:
        _CACHE["nc"] = _build()
    nc = _CACHE["nc"]

    maps = _in_maps(x, embeds, Wq, bq, Wk, bk, Wv, bv, Wo, bo)
    res = run_bass_kernel_spmd(nc, maps, core_ids=list(range(8)),
                               trace=_trace, tmpdir=_tmpdir)
    if _trace:
        _CACHE["last_exec_time_ns"] = res.exec_time_ns
    _CACHE["last_results"] = res

    out = np.empty((B, LQ, DIM), np.float32)
    for b in range(B):
        acc = (res.results[2 * b]["outT"].astype(np.float32)
               + res.results[2 * b + 1]["outT"].astype(np.float32))
        out[b] = acc.T
    return out


# revision 15
# speedup vs baseline: 1.2111x; 1.0937x over previous
"""Cross-attention kernel for 8 TRN2 NeuronCores.

Reference shapes: x [4, 2048, 1024], embeds [4, 2048, 1024],
Wq/Wk/Wv [1024, 1024] (+bias), Wo [1024, 1024] (+bias), H=16 heads, D=64.

Sharding: core c handles batch b = c//2 and head group hg = c%2 (8 heads,
attn-dim slice of 512).  Each core computes a partial output
outT_c [1024, 2048] (fp16); the host sums the two partials per batch
(row-parallel Wo all-reduce done at unshard time); bo is folded into the
even core's partial.

All matmul operands are fp16 (PSUM accumulates fp32).  Device dataflow:
  QT[m] = Wq_m^T @ xT      [128, 2048] per ad-tile m (4)   feature-major
  KT[m] = Wk_m^T @ embT    [128, 2048]
  V[t]  = embT_t^T @ Wv    [128, 520]  token-major, 8 heads x (64 cols + ones)
  per head h, q-half qh (1024 q):
    per lk-tile t: S = K_h-slice^T-form @ Q_h -> psum [128 lk, 1024 q]
                   E = exp(S/8)               -> sbuf fp16 (ACT, 1024-wide)
                   Cu[qc] += E_chunk^T @ [V_h|1]   psum [128 q, 65] per qc
    normalize: ctx_tok = Cu[:, :64] / Cu[:, 64]  (DVE divide, per-q scalar)
  transpose ctx_tok [q, ad] -> CT [ad, q] via XBAR DMA transpose (fp16)
  outT = Wo^T @ CT  + bo (even core)    -> fp16 out
Softmax skips the max-subtraction: scores ~ N(0,1), exp is safe in fp32.

Scheduling: engines execute their queues in order, so emission order is the
schedule.  The whole kernel is one "tick" loop over the 256 exp tiles
(qh, head, lk-tile): each tick emits the tile's S matmuls + exp, the ctx
matmuls of the tile two back (E-ring), any projection units whose deadline
arrived, and a budgeted trickle of remaining projection / output work.
This keeps the activation engine's exp stream (the ~266us floor) running
back-to-back while the PE (the ~274us floor) stays saturated.  The ctx
matmul is token-major (65-wide moving operand) because a matmul costs its
output free size: 8 heads x 16 q-chunks x 16 lk-passes x 65 halves the PE
cost vs the feature-major form.
"""

import sys

if "/opt/trn_rl_repo" not in sys.path:
    sys.path.insert(0, "/opt/trn_rl_repo")

import numpy as np

import concourse.bass as bass  # noqa: F401
import concourse.mybir as mybir
import concourse.tile as tile
from concourse import bacc
from concourse.bass_utils import run_bass_kernel_spmd

P = 128
B, LQ, LK, DIM = 4, 2048, 2048, 1024
H, D = 16, 64
ADC = 512          # per-core attention dim (8 heads x 64)
NHC = 8            # heads per core
SCALE = 1.0 / 8.0
F32 = mybir.dt.float32
F16 = mybir.dt.float16
EXP = mybir.ActivationFunctionType.Exp

K_T = DIM // P     # 8 contraction tiles for projections
M_AD = ADC // P    # 4 ad partition tiles (head pairs)
T_LK = LK // P     # 16 lk tiles
VW = D + 1         # 65: per-head V block width (64 cols + ones col)
VTW = NHC * VW     # 520: V block width per lk tile
LAG = 3            # ctx matmuls trail exp by this many ticks
ERING = 12         # E-ring depth (sbuf fp16 [128, 1024] slots)

_CACHE = {}


def _build():
    nc = bacc.Bacc("TRN2", target_bir_lowering=False, debug=False)

    xT = nc.dram_tensor("xT", [DIM, LQ], F16, kind="ExternalInput").ap()
    embT = nc.dram_tensor("embT", [DIM, LK], F16, kind="ExternalInput").ap()
    Wq = nc.dram_tensor("Wq", [DIM, ADC], F16, kind="ExternalInput").ap()
    Wk = nc.dram_tensor("Wk", [DIM, ADC], F16, kind="ExternalInput").ap()
    Wv = nc.dram_tensor("Wv", [DIM, ADC], F16, kind="ExternalInput").ap()
    Wo = nc.dram_tensor("Wo", [ADC, DIM], F16, kind="ExternalInput").ap()
    bq = nc.dram_tensor("bq", [P, M_AD], F32, kind="ExternalInput").ap()
    bk = nc.dram_tensor("bk", [P, M_AD], F32, kind="ExternalInput").ap()
    bvb = nc.dram_tensor("bvb", [P, ADC], F32, kind="ExternalInput").ap()
    bo = nc.dram_tensor("bo", [P, DIM // P], F32, kind="ExternalInput").ap()
    outT = nc.dram_tensor("outT", [DIM, LQ], F16, kind="ExternalOutput").ap()

    with tile.TileContext(nc) as tc:
        with tc.tile_pool(name="resident", bufs=1) as res:
            xs = res.tile([P, K_T, LQ], F16, name="xs")
            es = res.tile([P, K_T, LK], F16, name="es")
            wq_sb = res.tile([P, K_T, ADC], F16, name="wq")
            wk_sb = res.tile([P, K_T, ADC], F16, name="wk")
            wv_sb = res.tile([P, K_T, ADC], F16, name="wv")
            wo_sb = res.tile([P, M_AD, DIM], F16, name="wo")
            QT = [res.tile([P, LQ], F16, name=f"qt{m}") for m in range(M_AD)]
            KT = [res.tile([P, LK], F16, name=f"kt{m}") for m in range(M_AD)]
            V = res.tile([P, T_LK * VTW], F16, name="v")
            CT = [res.tile([P, LQ], F16, name=f"ct{m}") for m in range(M_AD)]
            bq_sb = res.tile([P, M_AD], F32, name="bq")
            bk_sb = res.tile([P, M_AD], F32, name="bk")
            bvb_sb = res.tile([P, ADC], F32, name="bvb")
            bo_sb = res.tile([P, DIM // P], F32, name="bo")

            # ---- input DMAs (SP queue; loads only, never block) ----
            # Ordered so the first S matmul (needs K pair 0 chunk 0 + Q pair
            # 0 q-half 0) can launch as early as possible.
            nc.sync.dma_start(bq_sb[:], bq[:])
            nc.sync.dma_start(bk_sb[:], bk[:])
            nc.sync.dma_start(bo_sb[:], bo[:])
            nc.sync.dma_start(bvb_sb[:], bvb[:])
            nc.sync.dma_start(wk_sb[:], Wk.rearrange("(k p) n -> p k n", p=P))
            nc.sync.dma_start(wq_sb[:], Wq.rearrange("(k p) n -> p k n", p=P))
            embT_kp = embT.rearrange("(k p) n -> p k n", p=P)
            xT_kp = xT.rearrange("(k p) n -> p k n", p=P)
            nc.sync.dma_start(es[:, :, 0:512], embT_kp[:, :, 0:512])
            nc.sync.dma_start(xs[:, :, 0:512], xT_kp[:, :, 0:512])
            nc.sync.dma_start(xs[:, :, 512:1024], xT_kp[:, :, 512:1024])
            nc.sync.dma_start(wv_sb[:], Wv.rearrange("(k p) n -> p k n", p=P))
            for n in range(1, 4):
                nc.sync.dma_start(es[:, :, n * 512:(n + 1) * 512],
                                  embT_kp[:, :, n * 512:(n + 1) * 512])
            nc.sync.dma_start(xs[:, :, 1024:2048], xT_kp[:, :, 1024:2048])
            nc.sync.dma_start(wo_sb[:], Wo.rearrange("(k p) n -> p k n", p=P))

            # ones columns for the fused-denominator ctx matmul: preset the
            # whole V tile to 1.0; V-proj bias-add overwrites the 64-wide
            # value blocks and leaves column 64 of each head block intact.
            nc.gpsimd.memset(V[:], 1.0)

            with tc.tile_pool(name="pj", bufs=2, space="PSUM") as pjp, \
                 tc.tile_pool(name="sw", bufs=2, space="PSUM") as swp, \
                 tc.tile_pool(name="cp", bufs=1, space="PSUM") as cpp, \
                 tc.tile_pool(name="ep", bufs=ERING) as epp, \
                 tc.tile_pool(name="ctok", bufs=2) as ctokp, \
                 tc.tile_pool(name="rcp", bufs=2) as rcpp, \
                 tc.tile_pool(name="os", bufs=4) as osp:

                # ---------- emission helpers ----------
                def emit_kproj(m, n):
                    ps = pjp.tile([P, 512], F32, name="pp")
                    for k in range(K_T):
                        nc.tensor.matmul(
                            ps[:], wk_sb[:, k, m * P:(m + 1) * P],
                            es[:, k, n * 512:(n + 1) * 512],
                            start=(k == 0), stop=(k == K_T - 1))
                    nc.vector.tensor_scalar_add(
                        KT[m][:, n * 512:(n + 1) * 512], ps[:],
                        bk_sb[:, m:m + 1])

                def emit_qproj(m, n):
                    ps = pjp.tile([P, 512], F32, name="pp")
                    for k in range(K_T):
                        nc.tensor.matmul(
                            ps[:], wq_sb[:, k, m * P:(m + 1) * P],
                            xs[:, k, n * 512:(n + 1) * 512],
                            start=(k == 0), stop=(k == K_T - 1))
                    nc.vector.tensor_scalar_add(
                        QT[m][:, n * 512:(n + 1) * 512], ps[:],
                        bq_sb[:, m:m + 1])

                def emit_vproj(t):
                    ps = pjp.tile([P, 512], F32, name="pp")
                    for k in range(K_T):
                        nc.tensor.matmul(
                            ps[:], es[:, k, t * P:(t + 1) * P],
                            wv_sb[:, k, :],
                            start=(k == 0), stop=(k == K_T - 1))
                    vdst = V[:, t * VTW:(t + 1) * VTW].rearrange(
                        "p (a b) -> p a b", b=VW)
                    nc.vector.tensor_tensor(
                        vdst[:, :, 0:D],
                        ps[:].rearrange("p (a b) -> p a b", b=D),
                        bvb_sb[:].rearrange("p (a b) -> p a b", b=D),
                        op=mybir.AluOpType.add)

                def emit_outproj(d, qn):
                    po = pjp.tile([P, 512], F32, name="pp")
                    for ch in range(M_AD):
                        nc.tensor.matmul(
                            po[:], wo_sb[:, ch, d * P:(d + 1) * P],
                            CT[ch][:, qn * 512:(qn + 1) * 512],
                            start=(ch == 0), stop=(ch == M_AD - 1))
                    ot = osp.tile([P, 512], F16, name="ot")
                    nc.vector.tensor_scalar_add(ot[:], po[:],
                                                bo_sb[:, d:d + 1])
                    nc.sync.dma_start(
                        outT[d * P:(d + 1) * P, qn * 512:(qn + 1) * 512],
                        ot[:])

                # ---------- deferred work with deadlines ----------
                # Units: (deadline_tick, avail_tick, est_ns, fn).  Forced
                # when their deadline tick arrives (just before that tick's
                # S matmuls need the result); otherwise trickled in whenever
                # the emitted-PE-work clock lags the projected ACT clock, so
                # the exp stream is never starved by front-loaded PE work.
                PROJ_NS = 1707   # 8 passes x 512 cols
                OUT_NS = 853     # 4 passes x 512 cols
                S_NS = 427
                C_NS = 217
                EXP_NS = 1038
                ACT_START = 15000.0
                work = []
                for m in range(1, M_AD):
                    t0 = 32 * m
                    work.append((t0 - 1, 0, PROJ_NS,
                                 lambda m=m: emit_qproj(m, 0)))
                    work.append((t0 - 1, 0, PROJ_NS,
                                 lambda m=m: emit_qproj(m, 1)))
                    for n in range(4):
                        work.append((t0 + 4 * n - 1, 0, PROJ_NS,
                                     lambda m=m, n=n: emit_kproj(m, n)))
                for m in range(M_AD):
                    t0 = 128 + 32 * m
                    work.append((t0 - 1, 16, PROJ_NS,
                                 lambda m=m: emit_qproj(m, 2)))
                    work.append((t0 - 1, 16, PROJ_NS,
                                 lambda m=m: emit_qproj(m, 3)))
                for d in range(DIM // P):
                    for qn in range(2):
                        work.append((255, 134, OUT_NS,
                                     lambda d=d, qn=qn: emit_outproj(d, qn)))
                work.sort(key=lambda w: w[0])

                clk = {"pe": 0.0}

                def run_unit(i):
                    _, _, ns, fn = work.pop(i)
                    fn()
                    clk["pe"] += ns

                # ---------- startup projections ----------
                emit_kproj(0, 0)
                emit_qproj(0, 0)
                emit_qproj(0, 1)
                clk["pe"] += 3 * PROJ_NS

                # ---------- global tick loop ----------
                ticks = [(qh, m, hh, t)
                         for qh in range(2)
                         for m in range(M_AD)
                         for hh in range(2)
                         for t in range(T_LK)]
                ering = [None] * ERING
                head_state = {}  # head index (g // T_LK) -> state dict
                cur = {"ctok": None, "qh": -1}
                vdone = [False] * T_LK

                def cuv(cu, qc):
                    off = (qc // 4) * 512 + (qc % 4) * VW
                    return cu[:, off:off + VW]

                def emit_ctx(gc):
                    # ctx matmuls for global tile gc; head 0 lazily emits the
                    # V projection for the lk-tile it is about to consume
                    hs = head_state[gc // T_LK]
                    t = gc % T_LK
                    if gc // T_LK == 0 and not vdone[t]:
                        emit_vproj(t)
                        vdone[t] = True
                        clk["pe"] += PROJ_NS
                    if hs["cu"] is None:
                        hs["cu"] = cpp.tile([P, 1024], F32, name="cu")
                    et = ering[gc % ERING]
                    cu = hs["cu"]
                    voff = t * VTW + hs["h"] * VW
                    for qc in range(8):
                        # start=True lazily zeroes the whole 2KB psum bank,
                        # so only the first matmul touching each bank sets it
                        nc.tensor.matmul(
                            cuv(cu, qc),
                            et[:, qc * P:(qc + 1) * P],
                            V[:, voff:voff + VW],
                            start=(t == 0 and qc % 4 == 0),
                            stop=(t == T_LK - 1))
                    clk["pe"] += C_NS
                    if t == T_LK - 1:
                        finish_head(hs)
                        del head_state[gc // T_LK]

                def finish_head(hs):
                    # normalize into ctok; at pair end, XBAR-transpose
                    cu, ctok, h = hs["cu"], hs["ctok"], hs["h"]
                    rcp = rcpp.tile([P, 8], F32, name="rcp")
                    for half in range(2):
                        dn = cu[:, half * 512:half * 512 + 4 * VW].rearrange(
                            "p (a b) -> p a b", b=VW)[:, :, D:D + 1]
                        nc.vector.reciprocal(
                            rcp[:, half * 4:(half + 1) * 4].rearrange(
                                "p (a b) -> p a b", b=1), dn)
                    for qc in range(8):
                        nc.vector.tensor_scalar(
                            ctok[:, qc, h * D:(h + 1) * D],
                            cuv(cu, qc)[:, 0:D],
                            rcp[:, qc:qc + 1], None,
                            op0=mybir.AluOpType.mult)
                    if hs["hh"] == 1:
                        m, q0 = hs["m"], hs["q0"]
                        for qc in range(8):
                            nc.sync.dma_start_transpose(
                                CT[m][:, q0 + qc * P:q0 + (qc + 1) * P],
                                ctok[:, qc, m * P:(m + 1) * P])

                cnext = [0]  # next global tile whose ctx matmuls are pending

                for g, (qh, m, hh, t) in enumerate(ticks):
                    if t == 0:
                        if qh != cur["qh"]:
                            cur["qh"] = qh
                            cur["ctok"] = ctokp.tile(
                                [P, LQ // P // 2, ADC], F16, name="ctok")
                        head_state[g // T_LK] = {
                            "h": 2 * m + hh, "m": m, "hh": hh,
                            "q0": qh * 1024, "gbase": g,
                            "cu": None, "ctok": cur["ctok"],
                        }
                    ro = hh * D
                    q0 = qh * 1024

                    # forced work whose deadline arrived (feeds this tick's S)
                    while work and work[0][0] <= g:
                        run_unit(0)
                    # K chunks for pair 0 stream in with the embT DMAs
                    if m == 0 and hh == 0 and qh == 0 and t in (4, 8, 12):
                        emit_kproj(0, t // 4)
                        clk["pe"] += PROJ_NS
                    # E-ring pressure: the slot exp(g) writes must have been
                    # consumed; emit those ctx matmuls first
                    while cnext[0] <= g - ERING + 2:
                        emit_ctx(cnext[0])
                        cnext[0] += 1

                    # S matmuls for this tick's lk-tile
                    sw = swp.tile([P, 1024], F32, name="sw")
                    for nn in range(2):
                        nc.tensor.matmul(
                            sw[:, nn * 512:(nn + 1) * 512],
                            KT[m][ro:ro + D, t * P:(t + 1) * P],
                            QT[m][ro:ro + D,
                                  q0 + nn * 512:q0 + (nn + 1) * 512],
                            start=True, stop=True)
                    et = epp.tile([P, 1024], F16, name="et")
                    nc.scalar.activation(et[:], sw[:], EXP, scale=SCALE)
                    ering[g % ERING] = et
                    clk["pe"] += S_NS

                    # pace the rest against the projected ACT clock
                    act_clk = ACT_START + (g + 1) * EXP_NS
                    while cnext[0] <= g - 2 and clk["pe"] < act_clk:
                        emit_ctx(cnext[0])
                        cnext[0] += 1
                    progress = True
                    while progress and clk["pe"] < act_clk:
                        progress = False
                        for i in range(len(work)):
                            if work[i][1] <= g:
                                run_unit(i)
                                progress = True
                                break
                        if not progress and cnext[0] <= g - 2:
                            emit_ctx(cnext[0])
                            cnext[0] += 1
                            progress = True

                # ---------- tail ----------
                while cnext[0] < 256:
                    emit_ctx(cnext[0])
                    cnext[0] += 1
                while work:
                    _, _, _, fn = work.pop(0)
                    fn()
                for d in range(DIM // P):
                    emit_outproj(d, 2)
                    emit_outproj(d, 3)

    nc.compile()
    return nc


def _in_maps(x, embeds, Wq, bq, Wk, bk, Wv, bv, Wo, bo):
    h = np.float16
    f = np.float32
    maps = []
    for c in range(8):
        b, hg = c // 2, c % 2
        s = slice(hg * ADC, (hg + 1) * ADC)
        bo_c = bo if hg == 0 else np.zeros_like(bo)
        maps.append({
            "xT": np.ascontiguousarray(x[b].T, dtype=h),
            "embT": np.ascontiguousarray(embeds[b].T, dtype=h),
            "Wq": np.ascontiguousarray(Wq[:, s], dtype=h),
            "Wk": np.ascontiguousarray(Wk[:, s], dtype=h),
            "Wv": np.ascontiguousarray(Wv[:, s], dtype=h),
            "Wo": np.ascontiguousarray(Wo[s, :], dtype=h),
            "bq": np.ascontiguousarray(bq[s].reshape(M_AD, P).T, dtype=f),
            "bk": np.ascontiguousarray(bk[s].reshape(M_AD, P).T, dtype=f),
            "bvb": np.ascontiguousarray(np.tile(bv[s], (P, 1)), dtype=f),
            "bo": np.ascontiguousarray(
                bo_c.reshape(DIM // P, P).T, dtype=f),
        })
    return maps


def kernel(x, embeds, Wq, bq, Wk, bk, Wv, bv, Wo, bo, _trace=False,
           _tmpdir=None):
    x = np.asarray(x); embeds = np.asarray(embeds)
    Wq = np.asarray(Wq); bq = np.asarray(bq)
    Wk = np.asarray(Wk); bk = np.asarray(bk)
    Wv = np.asarray(Wv); bv = np.asarray(bv)
    Wo = np.asarray(Wo); bo = np.asarray(bo)

    if "nc" not in _CACHE:
        _CACHE["nc"] = _build()
    nc = _CACHE["nc"]

    maps = _in_maps(x, embeds, Wq, bq, Wk, bk, Wv, bv, Wo, bo)
    res = run_bass_kernel_spmd(nc, maps, core_ids=list(range(8)),
                               trace=_trace, tmpdir=_tmpdir)
    if _trace:
        _CACHE["last_exec_time_ns"] = res.exec_time_ns
    _CACHE["last_results"] = res

    out = np.empty((B, LQ, DIM), np.float32)
    for b in range(B):
        acc = (res.results[2 * b]["outT"].astype(np.float32)
               + res.results[2 * b + 1]["outT"].astype(np.float32))
        out[b] = acc.T
    return out
